# revision 137
# baseline (speedup 1.0000x reference)
"""Trainium2 Bass kernel for nn_Encoding (vq_codebook).

Math (per batch b):
    xf = x[b].reshape(C, N).T                      # (N tokens, C)
    sl2[n,k] = scale[k] * (|xf_n|^2 - 2 xf_n.c_k + |c_k|^2)
    w = softmax_k(sl2)                             # max-subtract skipped: sl2 in (-600, -0.18]
    out[b] = w.T @ xf - (sum_n w)[:,None] * codewords

Sharding: data-parallel over batch B=32 -> 4 batches per core on 8 cores.

Per-core dataflow (units of 2048 tokens; last batch ends with 2x1024
to shorten the pipeline drain; 10 units/core, 3-deep software pipeline):
  - x loaded in natural (c-partition, token-free) layout, 2 half DMAs/unit
    on the SP queue; unit 0 streams both c-halves in parallel (SP + ACT
    queues) so PE starts ~3.5us in.  Small constants ride the ACT queue;
    a_s/scp ride the idle Pool SWDGE queue so mm1/fold aren't const-gated.
  - |x|^2 on the Pool engine in natural layout: sq_h = x_h*x_h per c-half,
    ssum = sq_1 + sq_2 (bf16).  scale_k * sum_c x^2 folds into the score
    PSUM via 4 group matmuls with a bf16 s_hi scale-pattern stationary; the
    bf16 residual s_lo rides the exp bias as s_lo*E[|x|^2] (no transpose,
    no DRAM bounce).
  - mm1: psum_sl2 (128 = 4 groups x 32 codes, 512 tokens) accumulates
    A = -2*scale*cw against streamed x (f32r, 1 cyc/row), one 32-col group
    per 512-token group, interleaved with the transposes; fold stops last.
  - x is cast to bf16 (split ACT/Pool/DVE) and PE is_transpose matmuls
    (1 cyc/row) build xT tiles in bf16 PSUM (4 chunks = 1 bank); one
    ACT/DVE evacuation per bank writes the bf16 xT (unit 0 skips the cast
    and transposes f32r so fill isn't cast-gated).
  - One ACT exp over (128, 512) with per-partition bias
    scale*(|c_k|^2) + s_lo*256.
  - Softmax denominators: PE matmul with group-indicator lhsT -> (4, 512);
    DVE reciprocal (bf16 out, uniform across codes per token so the
    rounding cancels); PE matmul broadcasts back to (128, 512); DVE
    multiply normalizes -> w (bf16).
  - PE transposes w into (token, code) tiles; mm2 (w stationary, bf16 xT
    moving) accumulates out (32, 258) per batch; wsum rides col 256 via
    the ones column of xT.
  - Final: one DVE scalar_tensor_tensor: out = cw*(-wsum) + wx; DMA out
    on the SP queue (idle at batch ends).
"""

import numpy as np
from contextlib import ExitStack

import concourse.bass as bass
import concourse.bacc as bacc
import concourse.mybir as mybir
import concourse.tile as tile
from concourse.bass_utils import run_bass_kernel_spmd

F32 = mybir.dt.float32
F32R = mybir.dt.float32r
BF16 = mybir.dt.bfloat16
ALU = mybir.AluOpType
ACTF = mybir.ActivationFunctionType

N_CORES = 8
B, C, K = 32, 256, 32
HW = 64 * 64            # 4096 tokens per batch
BL = B // N_CORES       # batches per core
UNIT = 2048             # tokens per unit
UNITS = BL * HW // UNIT  # 8 units per core
NGRP = 4                # 512-token groups per unit
GTOK = 512              # tokens per group
NCHUNK = 16             # 128-token chunks per unit


def build_module(bl=BL):
    nc = bacc.Bacc(None)
    units = bl * HW // UNIT

    x_d = nc.dram_tensor("x", (bl, 2, 128, HW), F32R, kind="ExternalInput")
    a_d = nc.dram_tensor("A", (2, 4, 128, 128), F32R, kind="ExternalInput")
    scp_d = nc.dram_tensor("SCP", (4, 128, 128), BF16, kind="ExternalInput")
    bias_d = nc.dram_tensor("BIASB", (128, 1), F32, kind="ExternalInput")
    gs_d = nc.dram_tensor("GS", (128, 4), F32R, kind="ExternalInput")
    gb_d = nc.dram_tensor("GB", (4, 128), F32, kind="ExternalInput")
    cw_d = nc.dram_tensor("CWD", (32, 256), F32, kind="ExternalInput")
    onz_d = nc.dram_tensor("ONZ", (128, 32), F32, kind="ExternalInput")
    idt_d = nc.dram_tensor("IDT", (128, 128), F32R, kind="ExternalInput")
    out_d = nc.dram_tensor("out", (bl, 32, 256), F32, kind="ExternalOutput")

    with tile.TileContext(nc) as tc, ExitStack() as ctx:
        sb = ctx.enter_context(tc.tile_pool(name="sb", bufs=3))
        cp = ctx.enter_context(tc.tile_pool(name="consts", bufs=1))
        ps_big = ctx.enter_context(tc.tile_pool(name="ps_big", bufs=2, space="PSUM"))
        ps_sm = ctx.enter_context(tc.tile_pool(name="ps_sm", bufs=1, space="PSUM"))
        ps_xt = ctx.enter_context(tc.tile_pool(name="ps_xt", bufs=3, space="PSUM"))
        ps_wtt = ctx.enter_context(tc.tile_pool(name="ps_wtt", bufs=1, space="PSUM"))
        ps_wx = ctx.enter_context(tc.tile_pool(name="ps_wx", bufs=1, space="PSUM"))

        # constants ride the ACT DMA queue so unit-0 x loads start on SP at t=0;
        # idt/a_s first: the earliest PE work (transposes, mm1) needs them
        def c(shape, dram, tag, dt=F32):
            t = cp.tile(shape, dt, tag=tag)
            nc.scalar.dma_start(t[:], dram[:])
            return t

        # unit schedule: (batch, t0, ntok, first_of_batch, last_of_batch);
        # small edge units shrink pipeline fill and drain
        sched = []
        for b_ in range(bl):
            if b_ == bl - 1:
                parts = [2048, 1024, 1024]
            else:
                parts = [2048, 2048]
            t0 = 0
            for i, ntok in enumerate(parts):
                sched.append((b_, t0, ntok, i == 0, i == len(parts) - 1))
                t0 += ntok

        idt_s = c([128, 128], idt_d, "idt", F32R)
        idtb_s = cp.tile([128, 128], BF16, tag="idtb")
        nc.gpsimd.tensor_copy(idtb_s[:], idt_s[:].bitcast(F32))
        # unit-0 second c-half rides the ACT queue right after idt so both
        # halves of the first unit stream in parallel with SP (two DMAs so
        # the first cc1 transposes start as soon as the first half lands)
        NT0 = sched[0][2]
        H0 = NT0 // 2
        xn0 = sb.tile([128, 2 * UNIT], F32R, tag="xn")
        nc.scalar.dma_start(xn0[:, NT0:NT0 + H0], x_d[0, 1, :, 0:H0])
        nc.scalar.dma_start(xn0[:, NT0 + H0:2 * NT0], x_d[0, 1, :, H0:NT0])
        a_s = cp.tile([128, 8, 128], F32R, tag="a")
        nc.gpsimd.dma_start(a_s[:], a_d[:].rearrange("c g p m -> p (c g) m"))
        scp_s = cp.tile([128, 4, 128], BF16, tag="scp")
        nc.gpsimd.dma_start(scp_s[:], scp_d[:].rearrange("g p m -> p g m"))
        bias_s = c([128, 1], bias_d, "bias")
        gs_s = c([128, 4], gs_d, "gs", F32R)
        gb_s = c([4, 128], gb_d, "gb")
        gbb_s = cp.tile([4, 128], BF16, tag="gbb")
        nc.gpsimd.tensor_copy(gbb_s[:], gb_s[:])
        cw_s = c([32, 256], cw_d, "cw")
        onz_s = c([128, 32], onz_d, "onz")

        pwx = {}

        def stage_a(u):
            """Load x, build xT + |x|^2 (Pool), run mm1 (+ssum fold) into psum_sl2."""
            b_, t0, ntok, first_b, last_b = sched[u]
            gtok = ntok // 4
            nchunk = ntok // 128
            if u == 0:
                # h2 already streaming on the ACT queue (see const section);
                # h1 in two quarters on SP so transposes start ~1.6us sooner
                xn = xn0
                for p0, p1 in ((0, 512), (512, 1024), (1024, ntok)):
                    nc.sync.dma_start(
                        xn[:, p0:p1], x_d[b_, 0, :, t0 + p0:t0 + p1])
            else:
                xn = sb.tile([128, 2 * UNIT], F32R, tag="xn")
                nc.sync.dma_start(xn[:, 0:ntok], x_d[b_, 0, :, t0:t0 + ntok])
                nc.sync.dma_start(
                    xn[:, ntok:2 * ntok], x_d[b_, 1, :, t0:t0 + ntok])

            # |x|^2 in natural layout on Pool: ssum[c%128, t] holds
            # x[c,t]^2 + x[c+128,t]^2; PE folds sum_c via scale-pattern matmuls.
            # ssum is bf16 so the fold matmul gets a hardware-rounded moving
            # operand (engines cannot emit f32r).
            xnf = xn[:].bitcast(F32)
            sq1 = sb.tile([128, UNIT], F32, tag="sq1")
            nc.gpsimd.tensor_tensor(
                sq1[:, 0:ntok], xnf[:, 0:ntok], xnf[:, 0:ntok], ALU.mult)
            sq2 = sb.tile([128, UNIT], F32, tag="sq2")
            nc.gpsimd.tensor_tensor(
                sq2[:, 0:ntok], xnf[:, ntok:2 * ntok], xnf[:, ntok:2 * ntok],
                ALU.mult)
            ssum = sb.tile([128, UNIT], BF16, tag="ssum")
            with nc.allow_low_precision(reason="|x|^2 partials to bf16; error averages out over 4096 tokens"):
                nc.gpsimd.tensor_tensor(
                    ssum[:, 0:ntok], sq1[:, 0:ntok], sq2[:, 0:ntok], ALU.add)

            # bf16 cast of x for the transposes (mm1 and the squares keep fp32);
            # the first two (small) units transpose f32r so fill isn't cast-gated
            if u >= 1:
                xnb = sb.tile([128, 2 * UNIT], BF16, tag="xnb")
                c1, c2 = 3 * ntok // 4, 5 * ntok // 4
                nc.scalar.copy(xnb[:, 0:c1], xnf[:, 0:c1])
                nc.gpsimd.tensor_copy(xnb[:, c1:c2], xnf[:, c1:c2])
                nc.vector.tensor_copy(xnb[:, c2:2 * ntok], xnf[:, c2:2 * ntok])
            else:
                xnb = None

            xT = sb.tile([128, NCHUNK * 258], BF16, tag="xT")
            # per chunk: col 256 = ones (mm2 col 256 accumulates wsum),
            # col 257 = zeros (pad to even moving-dim for the matmul).
            nc.vector.tensor_copy(
                xT[:, 0:nchunk * 258].rearrange("p (j c) -> p j c", c=258)[:, :, 256:258],
                onz_s[:, 0:2 * nchunk].rearrange("p (j c) -> p j c", c=2))

            psl2 = ps_big.tile([128, 512], F32, tag="big")

            def mm1(i, first=False):
                g, cc = i // 2, i % 2
                nc.tensor.matmul(
                    psl2[:, 0:gtok],
                    a_s[:, cc * 4 + g, :],
                    xn[:, cc * ntok + g * gtok:cc * ntok + (g + 1) * gtok],
                    start=first, stop=False, skip_group_check=True,
                )

            # bf16 PSUM transpose tiles pack 4 chunks per bank (2KB/part), so
            # one evacuation covers 4 chunks; f32r (unit 0) keeps 2 chunks
            cpt = 4 if xnb is not None else 2   # chunks per xtp tile
            mm1_per_iter = 8 // (nchunk // cpt)
            tsrc, tid = (xnb, idtb_s) if xnb is not None else (xn, idt_s)
            for j2 in range(nchunk // cpt):
                xtp = ps_xt.tile(
                    [128, 256 * cpt], BF16 if xnb is not None else F32R, tag="xt")
                for h in range(cpt):
                    j = cpt * j2 + h
                    for cc in (0, 1):
                        nc.tensor.transpose(
                            xtp[:, h * 256 + cc * 128:h * 256 + cc * 128 + 128],
                            tsrc[:, cc * ntok + j * 128:cc * ntok + j * 128 + 128],
                            tid[:],
                        )
                for q in range(mm1_per_iter):
                    i = j2 * mm1_per_iter + q
                    mm1(i, first=(i == 0))
                # one evacuation per PSUM tile, alternating ACT/DVE
                dst = xT[:, 0:nchunk * 258].rearrange(
                    "p (j c) -> p j c", c=258)[:, cpt * j2:cpt * j2 + cpt, 0:256]
                src = xtp[:] if xnb is not None else xtp[:].bitcast(F32)
                src = src.rearrange("p (h c) -> p h c", c=256)
                if j2 % 2 == 0:
                    nc.vector.tensor_copy(dst, src)
                else:
                    nc.scalar.copy(dst, src)

            # |x|^2 fold: bf16 s_hi stationary; the bf16 residual s_lo rides
            # the exp bias as s_lo*E[|x|^2] (only zero-mean s_lo*(v-256) is
            # dropped, same order as the accepted bf16 ssum noise)
            for g in range(NGRP):
                nc.tensor.matmul(
                    psl2[:, 0:gtok], scp_s[:, g, :],
                    ssum[:, g * gtok:(g + 1) * gtok],
                    start=False, stop=(g == NGRP - 1), skip_group_check=True,
                )
            return dict(psl2=psl2, xT=xT, b=b_, ntok=ntok,
                        first_b=first_b, last_b=last_b)

        def stage_b(st):
            """softmax + mm2 + (end of batch) final subtract + store."""
            psl2, xT, b_ = st["psl2"], st["xT"], st["b"]
            ntok, first_b, last_b = st["ntok"], st["first_b"], st["last_b"]
            gtok = ntok // 4
            nchunk = ntok // 128
            nsl = gtok // 128
            # half-pipelined softmax tail: recip/pR/wt/wtT/wtTs flow in
            # column halves so PE work starts while half 2 is still going
            e = sb.tile([128, 512], F32R, tag="e")
            ps4 = ps_sm.tile([4, 512], F32, tag="sm")
            r4 = sb.tile([4, 512], BF16, tag="r4")
            pR = ps_big.tile([128, 512], F32, tag="big")
            wt = sb.tile([128, 512], BF16, tag="wt")
            if first_b:
                pwx[b_] = ps_wx.tile([32, 258], F32, tag="wx", name="pwx")
            pwtT = ps_wtt.tile([128, 512], BF16, tag="wtt")
            wtTs = sb.tile([128, 512], BF16, tag="wtTs")
            nc.scalar.activation(e[:, 0:gtok], psl2[:, 0:gtok], ACTF.Exp, bias=bias_s[:])
            nc.tensor.matmul(ps4[:, 0:gtok], gs_s[:], e[:, 0:gtok])
            hcol = gtok // 2
            NH = 1
            for hh in range(NH):
                hc = gtok // NH
                cols = slice(hc * hh, hc * hh + hc)
                with nc.allow_low_precision(reason="1/denom in bf16: uniform across codes per token, cancels in softmax"):
                    nc.vector.reciprocal(r4[:, cols], ps4[:, cols])
                nc.tensor.matmul(pR[:, cols], gbb_s[:], r4[:, cols])
                nc.vector.tensor_tensor(
                    wt[:, cols], e[:, cols].bitcast(F32), pR[:, cols],
                    ALU.mult)
                for sl in range(hh * nsl // NH, (hh + 1) * nsl // NH):
                    # transpose of the full (128, 128) slice: column-block g
                    # of the result is wT for token-chunk j = nsl*g + sl.
                    nc.tensor.transpose(
                        pwtT[:, 128 * sl:128 * sl + 128],
                        wt[:, 128 * sl:128 * sl + 128],
                        idtb_s[:],
                    )
                nc.vector.tensor_copy(wtTs[:, cols], pwtT[:, cols])
                for j in range(nchunk):
                    if (j % nsl) // (nsl // NH) != hh:
                        continue
                    nc.tensor.matmul(
                        pwx[b_][:, 0:258],
                        wtTs[:, 128 * (j % nsl) + 32 * (j // nsl):128 * (j % nsl) + 32 * (j // nsl) + 32],
                        xT[:, 258 * j:258 * j + 258],
                        start=(first_b and hh == 0 and j == 0),
                        stop=(last_b and hh == NH - 1 and j == nchunk - 1),
                        skip_group_check=True,
                    )
            if last_b:
                outs = sb.tile([32, 256], F32, tag="outs")
                nc.vector.scalar_tensor_tensor(
                    out=outs[:], in0=cw_s[:], scalar=pwx[b_][:, 256:257],
                    in1=pwx[b_][:, 0:256], op0=ALU.mult, op1=ALU.add,
                )
                nc.sync.dma_start(out_d[b_], outs[:])
                del pwx[b_]

        nu = len(sched)
        sts = [stage_a(0), stage_a(1)]
        for u in range(2, nu):
            stage_b(sts[u - 2])
            sts.append(stage_a(u))
        stage_b(sts[-2])
        stage_b(sts[-1])

    nc.finalize()
    return nc


def host_constants(codewords, scale):
    cw = np.asarray(codewords, dtype=np.float32)
    sc = np.asarray(scale, dtype=np.float32)
    c_sq = (cw.astype(np.float64) ** 2).sum(-1).astype(np.float32)

    A = np.zeros((2, 4, 128, 128), np.float32)
    for cc in range(2):
        blk = (-2.0 * sc[None, :]) * cw[:, cc * 128:(cc + 1) * 128].T
        for g in range(4):
            A[cc, g, :, 32 * g:32 * g + 32] = blk

    import ml_dtypes
    BF = ml_dtypes.bfloat16
    sc_hi = sc.astype(BF)
    sc_lo = (sc.astype(np.float64) - sc_hi.astype(np.float64)).astype(np.float32)
    SCP = np.zeros((4, 128, 128), BF)
    BIASB = np.zeros((128, 1), np.float32)
    GS = np.zeros((128, 4), np.float32)
    GB = np.zeros((4, 128), np.float32)
    for g in range(4):
        SCP[g, :, 32 * g:32 * g + 32] = sc_hi[None, :]
        BIASB[32 * g:32 * g + 32, 0] = sc * c_sq + sc_lo * 256.0
        GS[32 * g:32 * g + 32, g] = 1.0
        GB[g, 32 * g:32 * g + 32] = 1.0

    return {
        "A": A, "SCP": SCP, "BIASB": BIASB, "GS": GS, "GB": GB,
        "CWD": np.ascontiguousarray(-cw),
        "ONZ": np.tile(np.array([1.0, 0.0], np.float32), (128, 16)),
        "IDT": np.eye(128, dtype=np.float32),
    }


_CACHE = {}


def kernel(x, codewords, scale):
    x = np.ascontiguousarray(np.asarray(x), dtype=np.float32)
    if "nc" not in _CACHE:
        _CACHE["nc"] = build_module()
    nc = _CACHE["nc"]
    consts = host_constants(codewords, scale)
    xs = x.reshape(B, 2, 128, HW)
    in_maps = []
    for i in range(N_CORES):
        m = dict(consts)
        m["x"] = np.ascontiguousarray(xs[BL * i:BL * (i + 1)])
        in_maps.append(m)
    res = run_bass_kernel_spmd(nc, in_maps, list(range(N_CORES)))
    out = np.concatenate([r["out"] for r in res.results], axis=0)
    return out.astype(np.float32)
